# revision 12
# baseline (speedup 1.0000x reference)
"""Channel-attention module kernel for 8 Trainium2 NeuronCores.

reference semantics (B=2, C=128, N=D*H*W=147456):
    q = x.reshape(B, C, N)
    energy = q @ q^T                  # [B, C, C]
    attn = softmax(rowmax(energy) - energy, axis=-1)
          = softmax(-energy, axis=-1)             (rowmax shift is a no-op)
    out = attn @ q
    return x + gamma * out

Sharding: sequence-parallel over N. Core r owns columns
[r*N/8, (r+1)*N/8) of q for both batches. Each core computes a partial
energy (contraction over its local n), ONE AllReduce sums both batches'
[C, C] energies across the 8 cores, each core then computes the softmax
redundantly and applies the attention to its local columns.

Precision/layout scheme (v3):
  - The host splits q into q = hi + lo with hi = bf16(q) and
    lo = bf16(q - hi) (~16 mantissa bits combined), and ships BOTH in a
    pre-transposed, tile-major layout A[b, p, t, c] = qT[b, t*128+p, c]
    so each [n=128, C] matmul operand tile is a plain column slice of a
    contiguous chunk DMA (4 KiB per partition line).
  - energy = Qhi Qhi^T + (Qhi Qlo^T) + (Qhi Qlo^T)^T, dropping the
    O(2^-16) lo*lo term: two bf16 matmul chains per batch (1 cyc/row
    each) instead of one fp32 chain (4 cyc/row) + fp32 transposes
    (2 cyc/row). Measured pipeline rel err 1.7e-3, same as the fp32
    phase-1 baseline (phase-2 bf16 dominates the error budget).
  - phase 2 needs q back in [C, n] layout: the hi tiles are transposed
    on the PE (bf16, 1 cyc/row), 8 per PSUM bank, and copied once per
    [128, 1024] group into resident bf16 chunks.
  - phase 2 folds the residual into the attention matrix
    (attn_s = gamma/Z * P + I; P's diagonal is exactly 0 because the
    energy diagonal ~ +N dominates), so out = attn_s @ q_hi in bf16.

Collective path (hw-measured): the first collective pays a ~60us ncfw
cold-start from its dispatch trigger, so a dummy warmup AllReduce with
NO input dependencies is dispatched as the first gpsimd instruction
(reading uninitialized dram — its value is never used). Both batches'
energies ride ONE warm AllReduce ([C, 2C]) whose input halves are
bounced as soon as each batch's accumulation finishes.
"""

import sys

sys.path.insert(0, "/opt/trn_rl_repo")

import numpy as np

B, C = 2, 128
D, H, W = 16, 96, 96
N = D * H * W  # 147456
NCORES = 8
NLOC = N // NCORES  # 18432
T = NLOC // C  # 144 n-tiles of 128 per batch
CHUNK = 2048
NCHUNK = NLOC // CHUNK  # 9
TPC = CHUNK // C  # 16 n-tiles per chunk
OTILE = 512

_compiled = {}


def _log(msg):
    import time as _t
    print(f"[kernel {_t.strftime('%H:%M:%S')}] {msg}", flush=True)


def _build():
    import concourse.bacc as bacc
    import concourse.tile as tile
    import concourse.mybir as mybir

    _log("build start")

    f32 = mybir.dt.float32
    f16 = mybir.dt.float16
    bf16 = mybir.dt.bfloat16
    nc = bacc.Bacc("TRN2", target_bir_lowering=False, debug=False,
                   num_devices=NCORES)

    hi_d = nc.dram_tensor("qhT", [B, C, T * C], bf16, kind="ExternalInput").ap()
    lo_d = nc.dram_tensor("qlT", [B, C, T * C], bf16, kind="ExternalInput").ap()
    g_d = nc.dram_tensor("gamma_col", [C, 1], f32, kind="ExternalInput").ap()
    id_d = nc.dram_tensor("ident", [C, C], f32, kind="ExternalInput").ap()
    idb_d = nc.dram_tensor("identb", [C, C], bf16, kind="ExternalInput").ap()
    o_d = nc.dram_tensor("out", [B, C, NLOC], f16, kind="ExternalOutput").ap()

    with tile.TileContext(nc) as tc:
        with (
            tc.tile_pool(name="hring", bufs=NCHUNK + 1) as hp,
            tc.tile_pool(name="lring", bufs=4) as lp,
            tc.tile_pool(name="xb16", bufs=B * NCHUNK) as xbp,
            tc.tile_pool(name="tps", bufs=2, space="PSUM") as tps,
            tc.tile_pool(name="eps", bufs=2, space="PSUM") as eps,
            tc.tile_pool(name="ops", bufs=3, space="PSUM") as ops,
            tc.tile_pool(name="fps", bufs=1, space="PSUM") as fps,
            tc.tile_pool(name="misc", bufs=1) as mp,
            tc.tile_pool(name="ost", bufs=3) as ostp,
            tc.tile_pool(name="dram", bufs=1, space="DRAM") as dramp,
        ):
            # Warm-up collective FIRST, with no input dependency: the value
            # is garbage and never read; its only job is to absorb the ~60us
            # ncfw cold-start while input DMAs and phase 1 run.
            w_in = dramp.tile([C, 1], f32, name="w_in")
            w_out = dramp.tile([C, 1], f32, name="w_out", addr_space="Shared")
            nc.gpsimd.collective_compute(
                "AllReduce", mybir.AluOpType.add,
                replica_groups=[list(range(NCORES))],
                ins=[w_in.opt()], outs=[w_out.opt()],
            )

            ident = mp.tile([C, C], f32, name="ident_sb")
            identb = mp.tile([C, C], bf16, name="identb_sb")
            nc.sync.dma_start(identb[:], idb_d[:])
            nc.sync.dma_start(ident[:], id_d[:])
            # first chunk split so the PE starts after a quarter-chunk
            ht0 = hp.tile([C, CHUNK], bf16, name="h_0_0", tag="h")
            nc.sync.dma_start(ht0[:, 0:512], hi_d[0, :, 0:512])
            lt0 = lp.tile([C, CHUNK], bf16, name="l_0_0", tag="l")
            nc.sync.dma_start(lt0[:, 0:512], lo_d[0, :, 0:512])
            nc.sync.dma_start(ht0[:, 512:CHUNK], hi_d[0, :, 512:CHUNK])
            nc.sync.dma_start(lt0[:, 512:CHUNK], lo_d[0, :, 512:CHUNK])
            gcol = mp.tile([C, 1], f32, name="gcol")
            nc.sync.dma_start(gcol[:], g_d[:])

            xb16 = [[xbp.tile([C, CHUNK], bf16, name=f"xb_{b}_{k}", tag="xb")
                     for k in range(NCHUNK)] for b in range(B)]

            # per-batch AllReduce buffers; dispatch order on the gpsimd
            # queue is [warmup, bounce0, AR0, bounce1, AR1, rb0, rb1] so
            # batch 1's bounce is not stuck behind AR0's readback
            ar_in = [dramp.tile([C, C], f32, name=f"ar_in{b}")
                     for b in range(B)]
            ar_out = [dramp.tile([C, C], f32, name=f"ar_out{b}",
                                 addr_space="Shared") for b in range(B)]
            e_red = mp.tile([C, 2 * C], f32, name="e_red")

            hkeep = {}  # live hi chunks of the current batch

            def emit_phase1_mms(b):
                e_main = eps.tile([C, C], f32, name=f"em{b}", tag="e")
                e_cross = eps.tile([C, C], f32, name=f"ec{b}", tag="e")
                for k in range(NCHUNK):
                    if b == 0 and k == 0:
                        ht, lt = ht0, lt0
                    else:
                        ht = hp.tile([C, CHUNK], bf16, name=f"h_{b}_{k}",
                                     tag="h")
                        nc.sync.dma_start(
                            ht[:], hi_d[b, :, k * CHUNK:(k + 1) * CHUNK])
                        lt = lp.tile([C, CHUNK], bf16, name=f"l_{b}_{k}",
                                     tag="l")
                        nc.sync.dma_start(
                            lt[:], lo_d[b, :, k * CHUNK:(k + 1) * CHUNK])
                    hkeep[k] = ht
                    if b == 0 and k == 0:
                        # consume the quarter-chunk first so the PE starts
                        # as early as possible during the DMA ramp
                        order = [("hh", j) for j in range(4)] \
                            + [("hl", j) for j in range(4)] \
                            + [p for j in range(4, TPC)
                               for p in (("hh", j), ("hl", j))]
                    else:
                        order = [p for j in range(TPC)
                                 for p in (("hh", j), ("hl", j))]
                    for kind, j in order:
                        t = k * TPC + j
                        hs = ht[:, j * C:(j + 1) * C]
                        if kind == "hh":
                            nc.tensor.matmul(e_main[:], hs, hs,
                                             start=(t == 0), stop=(t == T - 1))
                        else:
                            nc.tensor.matmul(e_cross[:], hs,
                                             lt[:, j * C:(j + 1) * C],
                                             start=(t == 0), stop=(t == T - 1))
                # E_partial = e_main + e_cross + e_cross^T
                ecr = mp.tile([C, C], f32, name=f"ecr{b}")
                nc.vector.tensor_copy(ecr[:], e_cross[:])
                tpc_ps = tps.tile([C, C], f32, name=f"tpc{b}", tag="tp")
                nc.tensor.transpose(tpc_ps[:], ecr[:], ident[:])
                e_sum = mp.tile([C, C], f32, name=f"esum{b}")
                nc.vector.tensor_tensor(e_sum[:], e_main[:], ecr[:],
                                        op=mybir.AluOpType.add)
                e_cat = mp.tile([C, C], f32, name=f"e_cat{b}")
                nc.vector.tensor_tensor(e_cat[:], e_sum[:], tpc_ps[:],
                                        op=mybir.AluOpType.add)
                # bounce the AllReduce input; SWDGE (gpsimd) so the HWDGE
                # FIFO of chunk loads isn't blocked
                nc.gpsimd.dma_start(ar_in[b][:], e_cat[:])
                nc.gpsimd.collective_compute(
                    "AllReduce", mybir.AluOpType.add,
                    replica_groups=[list(range(NCORES))],
                    ins=[ar_in[b].opt()], outs=[ar_out[b].opt()],
                )

            def emit_transposes(b):
                # hi tiles -> [C, n] bf16 resident chunks for phase 2;
                # emitted after the AR bounce so they fill the PE while
                # the collective path is busy.
                cp = 0
                for k in range(NCHUNK):
                    ht = hkeep[k]
                    for g in range(TPC // 8):
                        tp = tps.tile([C, 8 * C], bf16,
                                      name=f"tp_{b}_{k}_{g}", tag="tp")
                        for u in range(8):
                            j = g * 8 + u
                            nc.tensor.transpose(tp[:, u * C:(u + 1) * C],
                                                ht[:, j * C:(j + 1) * C],
                                                identb[:])
                        dst = xb16[b][k][:, g * 8 * C:(g + 1) * 8 * C]
                        if cp % 2 == 0:
                            nc.vector.tensor_copy(dst, tp[:])
                        else:
                            nc.scalar.copy(dst, tp[:])
                        cp += 1
                hkeep.clear()

            def emit_softmax_vec(b):
                # everything up to P_b = gamma/Z * exp(min-E) + I
                E_b = e_red[:, b * C:(b + 1) * C]
                mcol = mp.tile([C, 1], f32, name=f"mcol{b}")
                nc.vector.tensor_reduce(mcol[:], E_b, axis=mybir.AxisListType.X,
                                        op=mybir.AluOpType.min)
                P_b = mp.tile([C, C], f32, name=f"P{b}")
                zcol = mp.tile([C, 1], f32, name=f"zcol{b}")
                # P = exp(min_row - E), zcol = rowsum(P); exponents <= 0.
                # P's diagonal is exp(min - ~+147000) == 0 exactly.
                nc.scalar.activation(P_b[:], E_b,
                                     mybir.ActivationFunctionType.Exp,
                                     bias=mcol[:], scale=-1.0,
                                     accum_out=zcol[:])
                rz = mp.tile([C, 1], f32, name=f"rz{b}")
                nc.vector.reciprocal(rz[:], zcol[:])
                scol = mp.tile([C, 1], f32, name=f"scol{b}")
                nc.vector.tensor_tensor(scol[:], rz[:], gcol[:],
                                        op=mybir.AluOpType.mult)
                # attn_s = (gamma/Z) * P + I  -> matmul computes x + gamma*attn@q
                nc.vector.tensor_scalar_mul(P_b[:], P_b[:], scol[:])
                nc.vector.tensor_add(P_b[:], P_b[:], ident[:])
                return P_b

            def emit_softmax_fin(b, P_b, copy_eng):
                tp2 = tps.tile([C, C], f32, name=f"tpP{b}", tag="tp")
                nc.tensor.transpose(tp2[:], P_b[:], ident[:])
                attnT = mp.tile([C, C], bf16, name=f"attnT{b}")
                copy_eng(attnT[:], tp2[:])  # fp32 psum -> bf16
                return attnT

            def emit_apply_chunk(b, attnT, k, three_eng):
                ost = ostp.tile([C, CHUNK], f16, name=f"ost_{b}_{k}",
                                tag="ost")
                for j in range(CHUNK // OTILE):
                    op = ops.tile([C, OTILE], f32, name=f"op_{b}_{k}_{j}",
                                  tag="op")
                    nc.tensor.matmul(
                        op[:], attnT[:],
                        xb16[b][k][:, j * OTILE:(j + 1) * OTILE],
                        start=True, stop=True)
                    dst = ost[:, j * OTILE:(j + 1) * OTILE]
                    jj = k * (CHUNK // OTILE) + j
                    if jj % 2 == 0:
                        nc.vector.tensor_copy(dst, op[:])
                    else:
                        nc.scalar.copy(dst, op[:])
                nc.sync.dma_start(o_d[b, :, k * CHUNK:(k + 1) * CHUNK],
                                  ost[:])

            for b in range(B):
                emit_phase1_mms(b)   # ends with this batch's bounce + AR
                emit_transposes(b)   # PE work that overlaps the collective
            for b in range(B):
                nc.gpsimd.dma_start(e_red[:, b * C:(b + 1) * C],
                                    ar_out[b][:])

            # PE keep-warm fillers: the collective wait would otherwise idle
            # the PE long enough for DVFS to drop it out of the 2.4 GHz
            # p-state, roughly doubling every phase-2 matmul. Junk matmuls
            # (no consumers) hold the clock up until attn(b0) is ready.
            junk_ps = fps.tile([C, OTILE], f32, name="junk_ps")

            def fillers(n):
                for _ in range(n):
                    nc.tensor.matmul(junk_ps[:], identb[:],
                                     xb16[1][0][:, 0:OTILE],
                                     start=True, stop=True)

            fillers(70)
            P0 = emit_softmax_vec(0)
            attnT0 = emit_softmax_fin(0, P0, nc.vector.tensor_copy)
            fillers(6)
            for k in range(5):
                emit_apply_chunk(0, attnT0, k, three_eng=False)
            # batch 1 softmax rides the vector queue here so it executes
            # right as AR(b1) lands, between batch-0 output copies
            P1 = emit_softmax_vec(1)
            for k in range(5, NCHUNK):
                emit_apply_chunk(0, attnT0, k, three_eng=False)
            attnT1 = emit_softmax_fin(1, P1, nc.scalar.copy)
            fillers(5)
            for k in range(NCHUNK):
                emit_apply_chunk(1, attnT1, k, three_eng=True)

    _log("tile context done; bacc compile start")
    nc.compile()
    _log("bacc compile done")
    return nc


def _get_nc():
    if "nc" not in _compiled:
        _compiled["nc"] = _build()
    return _compiled["nc"]


def kernel(x, gamma, _trace=False, _tmpdir=None):
    import ml_dtypes
    from concourse import bass_utils

    bf16 = ml_dtypes.bfloat16
    x = np.ascontiguousarray(np.asarray(x), dtype=np.float32)
    gamma = np.asarray(gamma, dtype=np.float32)
    q = x.reshape(B, C, N)
    hi = q.astype(bf16)
    lo = (q - hi.astype(np.float32)).astype(bf16)
    # tile-major transposed layout: A[r][b, p, t, c] = qT[b, r*NLOC+t*128+p, c]
    Ahi = np.ascontiguousarray(
        hi.reshape(B, C, NCORES, T, C).transpose(2, 0, 4, 3, 1)
    ).reshape(NCORES, B, C, T * C)
    Alo = np.ascontiguousarray(
        lo.reshape(B, C, NCORES, T, C).transpose(2, 0, 4, 3, 1)
    ).reshape(NCORES, B, C, T * C)
    gcol = np.full((C, 1), gamma[0], dtype=np.float32)
    ident = np.eye(C, dtype=np.float32)
    identb = np.eye(C, dtype=bf16)

    in_maps = []
    for r in range(NCORES):
        in_maps.append({
            "qhT": Ahi[r],
            "qlT": Alo[r],
            "gamma_col": gcol,
            "ident": ident,
            "identb": identb,
        })

    nc = _get_nc()
    _log("launching run_bass_kernel_spmd")
    res = bass_utils.run_bass_kernel_spmd(
        nc, in_maps, core_ids=list(range(NCORES)), trace=_trace,
        tmpdir=_tmpdir)
    outs = [res.results[r]["out"] for r in range(NCORES)]
    full = np.concatenate(outs, axis=2).astype(np.float32)
    full = full.reshape(B, C, D, H, W)
    if _trace:
        return full.astype(np.float32, copy=False), res
    return full.astype(np.float32, copy=False)


# revision 13
# speedup vs baseline: 1.0202x; 1.0202x over previous
"""Channel-attention module kernel for 8 Trainium2 NeuronCores.

reference semantics (B=2, C=128, N=D*H*W=147456):
    q = x.reshape(B, C, N)
    energy = q @ q^T                  # [B, C, C]
    attn = softmax(rowmax(energy) - energy, axis=-1)
          = softmax(-energy, axis=-1)             (rowmax shift is a no-op)
    out = attn @ q
    return x + gamma * out

Sharding: sequence-parallel over N. Core r owns columns
[r*N/8, (r+1)*N/8) of q for both batches. Each core computes a partial
energy (contraction over its local n), per-batch AllReduces sum the tiny
[C, C] energies across the 8 cores, each core then computes the softmax
redundantly and applies the attention to its local columns.

Precision/layout scheme (v6):
  - The host splits q into q = hi + lo with hi = bf16(q) and
    lo = bf16(q - hi) (~16 mantissa bits combined), shipped in a
    pre-transposed tile-major layout A[b, p, t, c] = qT[b, t*128+p, c]
    so each [n=128, C] matmul operand tile is a plain column slice of a
    contiguous chunk DMA. energy = Qhi Qhi^T + (Qhi Qlo^T) +
    (Qhi Qlo^T)^T, dropping the O(2^-16) lo*lo term: two bf16 matmul
    chains per batch (1 cyc/row) instead of one fp32 chain (4 cyc/row).
    Measured pipeline rel err 1.7e-3 (phase-2 bf16 dominates).
  - phase 2 needs q in [C, n] layout: shipped as a THIRD host tensor
    (bf16, normal layout) rather than PE-transposing on device — the
    extra 9.4 MB of DMA streams through the otherwise-idle bandwidth
    window between the phase-1 loads and the output stores (HWDGE FIFO
    order guarantees it cannot delay the phase-1 loads), and it frees
    the PE of 288 transposes and DVE/ACT of 36 large PSUM copies.
  - phase 2 folds the residual into the attention matrix
    (attn_s = gamma/Z * P + I; P's diagonal is exactly 0), so
    out = attn_s @ q_hi in bf16, f16 out.

Collective path (hw-measured): the first collective pays a ~45-60us
ncfw cold-start from its dispatch trigger (trigger floor ~20us after
kernel start), so a dummy warmup AllReduce with NO input dependencies
is dispatched as the first gpsimd instruction (reading uninitialized
dram — its value is never used). The per-batch energy AllReduces then
run back-to-back on the warm CC engine; batch 0's softmax+apply hide
batch 1's AllReduce latency.
"""

import sys

sys.path.insert(0, "/opt/trn_rl_repo")

import numpy as np

B, C = 2, 128
D, H, W = 16, 96, 96
N = D * H * W  # 147456
NCORES = 8
NLOC = N // NCORES  # 18432
T = NLOC // C  # 144 n-tiles of 128 per batch
CHUNK = 2048
NCHUNK = NLOC // CHUNK  # 9
TPC = CHUNK // C  # 16 n-tiles per chunk
OTILE = 512

_compiled = {}


def _log(msg):
    import time as _t
    print(f"[kernel {_t.strftime('%H:%M:%S')}] {msg}", flush=True)


def _build():
    import concourse.bacc as bacc
    import concourse.tile as tile
    import concourse.mybir as mybir

    _log("build start")

    f32 = mybir.dt.float32
    f16 = mybir.dt.float16
    bf16 = mybir.dt.bfloat16
    nc = bacc.Bacc("TRN2", target_bir_lowering=False, debug=False,
                   num_devices=NCORES)

    hi_d = nc.dram_tensor("qhT", [B, C, T * C], bf16, kind="ExternalInput").ap()
    lo_d = nc.dram_tensor("qlT", [B, C, T * C], bf16, kind="ExternalInput").ap()
    xb_d = nc.dram_tensor("xb", [B, C, NLOC], bf16, kind="ExternalInput").ap()
    g_d = nc.dram_tensor("gamma_col", [C, 1], f32, kind="ExternalInput").ap()
    id_d = nc.dram_tensor("ident", [C, C], f32, kind="ExternalInput").ap()
    o_d = nc.dram_tensor("out", [B, C, NLOC], f16, kind="ExternalOutput").ap()

    with tile.TileContext(nc) as tc:
        with (
            tc.tile_pool(name="hring", bufs=4) as hp,
            tc.tile_pool(name="lring", bufs=4) as lp,
            tc.tile_pool(name="xb16", bufs=B * NCHUNK) as xbp,
            tc.tile_pool(name="eps", bufs=2, space="PSUM") as eps,
            tc.tile_pool(name="sps", bufs=2, space="PSUM") as sps,
            tc.tile_pool(name="ops", bufs=4, space="PSUM") as ops,
            tc.tile_pool(name="misc", bufs=1) as mp,
            tc.tile_pool(name="ost", bufs=3) as ostp,
            tc.tile_pool(name="dram", bufs=1, space="DRAM") as dramp,
        ):
            # Warm-up collective FIRST, with no input dependency: the value
            # is garbage and never read; its only job is to absorb the ncfw
            # cold-start while input DMAs and phase 1 run.
            w_in = dramp.tile([C, 1], f32, name="w_in")
            w_out = dramp.tile([C, 1], f32, name="w_out", addr_space="Shared")
            nc.gpsimd.collective_compute(
                "AllReduce", mybir.AluOpType.add,
                replica_groups=[list(range(NCORES))],
                ins=[w_in.opt()], outs=[w_out.opt()],
            )

            ident = mp.tile([C, C], f32, name="ident_sb")
            nc.sync.dma_start(ident[:], id_d[:])
            # first chunk split so the PE starts after a quarter-chunk
            ht0 = hp.tile([C, CHUNK], bf16, name="h_0_0", tag="h")
            nc.sync.dma_start(ht0[:, 0:512], hi_d[0, :, 0:512])
            lt0 = lp.tile([C, CHUNK], bf16, name="l_0_0", tag="l")
            nc.sync.dma_start(lt0[:, 0:512], lo_d[0, :, 0:512])
            nc.sync.dma_start(ht0[:, 512:CHUNK], hi_d[0, :, 512:CHUNK])
            nc.sync.dma_start(lt0[:, 512:CHUNK], lo_d[0, :, 512:CHUNK])
            gcol = mp.tile([C, 1], f32, name="gcol")
            nc.sync.dma_start(gcol[:], g_d[:])

            xb16 = [[xbp.tile([C, CHUNK], bf16, name=f"xb_{b}_{k}", tag="xb")
                     for k in range(NCHUNK)] for b in range(B)]

            # per-batch AllReduce buffers
            ar_in = [dramp.tile([C, C], f32, name=f"ar_in{b}")
                     for b in range(B)]
            ar_out = [dramp.tile([C, C], f32, name=f"ar_out{b}",
                                 addr_space="Shared") for b in range(B)]
            e_red = mp.tile([C, 2 * C], f32, name="e_red")

            def emit_phase1_mms(b):
                e_main = eps.tile([C, C], f32, name=f"em{b}", tag="e")
                e_cross = eps.tile([C, C], f32, name=f"ec{b}", tag="e")
                for k in range(NCHUNK):
                    if b == 0 and k == 0:
                        ht, lt = ht0, lt0
                    else:
                        ht = hp.tile([C, CHUNK], bf16, name=f"h_{b}_{k}",
                                     tag="h")
                        nc.sync.dma_start(
                            ht[:], hi_d[b, :, k * CHUNK:(k + 1) * CHUNK])
                        lt = lp.tile([C, CHUNK], bf16, name=f"l_{b}_{k}",
                                     tag="l")
                        nc.sync.dma_start(
                            lt[:], lo_d[b, :, k * CHUNK:(k + 1) * CHUNK])
                    if b == 0 and k == 0:
                        # consume the quarter-chunk first so the PE starts
                        # as early as possible during the DMA ramp
                        order = [("hh", j) for j in range(4)] \
                            + [("hl", j) for j in range(4)] \
                            + [p for j in range(4, TPC)
                               for p in (("hh", j), ("hl", j))]
                    else:
                        order = [p for j in range(TPC)
                                 for p in (("hh", j), ("hl", j))]
                    for kind, j in order:
                        t = k * TPC + j
                        hs = ht[:, j * C:(j + 1) * C]
                        if kind == "hh":
                            nc.tensor.matmul(e_main[:], hs, hs,
                                             start=(t == 0), stop=(t == T - 1))
                        else:
                            nc.tensor.matmul(e_cross[:], hs,
                                             lt[:, j * C:(j + 1) * C],
                                             start=(t == 0), stop=(t == T - 1))
                # E_partial = e_main + e_cross + e_cross^T
                ecr = mp.tile([C, C], f32, name=f"ecr{b}")
                nc.vector.tensor_copy(ecr[:], e_cross[:])
                tpc_ps = sps.tile([C, C], f32, name=f"tpc{b}", tag="s")
                nc.tensor.transpose(tpc_ps[:], ecr[:], ident[:])
                e_sum = mp.tile([C, C], f32, name=f"esum{b}")
                nc.vector.tensor_tensor(e_sum[:], e_main[:], ecr[:],
                                        op=mybir.AluOpType.add)
                e_cat = mp.tile([C, C], f32, name=f"e_cat{b}")
                nc.vector.tensor_tensor(e_cat[:], e_sum[:], tpc_ps[:],
                                        op=mybir.AluOpType.add)
                # bounce the AllReduce input; SWDGE (gpsimd) so the HWDGE
                # FIFO of chunk loads isn't blocked
                nc.gpsimd.dma_start(ar_in[b][:], e_cat[:])
                nc.gpsimd.collective_compute(
                    "AllReduce", mybir.AluOpType.add,
                    replica_groups=[list(range(NCORES))],
                    ins=[ar_in[b].opt()], outs=[ar_out[b].opt()],
                )

            def emit_softmax_vec(b):
                # everything up to P_b = gamma/Z * exp(min-E) + I
                E_b = e_red[:, b * C:(b + 1) * C]
                mcol = mp.tile([C, 1], f32, name=f"mcol{b}")
                nc.vector.tensor_reduce(mcol[:], E_b, axis=mybir.AxisListType.X,
                                        op=mybir.AluOpType.min)
                P_b = mp.tile([C, C], f32, name=f"P{b}")
                zcol = mp.tile([C, 1], f32, name=f"zcol{b}")
                # P = exp(min_row - E), zcol = rowsum(P); exponents <= 0.
                # P's diagonal is exp(min - ~+147000) == 0 exactly.
                nc.scalar.activation(P_b[:], E_b,
                                     mybir.ActivationFunctionType.Exp,
                                     bias=mcol[:], scale=-1.0,
                                     accum_out=zcol[:])
                rz = mp.tile([C, 1], f32, name=f"rz{b}")
                nc.vector.reciprocal(rz[:], zcol[:])
                scol = mp.tile([C, 1], f32, name=f"scol{b}")
                nc.vector.tensor_tensor(scol[:], rz[:], gcol[:],
                                        op=mybir.AluOpType.mult)
                # attn_s = (gamma/Z) * P + I  -> matmul computes x + gamma*attn@q
                nc.vector.tensor_scalar_mul(P_b[:], P_b[:], scol[:])
                nc.vector.tensor_add(P_b[:], P_b[:], ident[:])
                return P_b

            def emit_softmax_fin(b, P_b, copy_eng):
                tp2 = sps.tile([C, C], f32, name=f"tpP{b}", tag="s")
                nc.tensor.transpose(tp2[:], P_b[:], ident[:])
                attnT = mp.tile([C, C], bf16, name=f"attnT{b}")
                copy_eng(attnT[:], tp2[:])  # fp32 psum -> bf16
                return attnT

            def emit_apply_chunk(b, attnT, k):
                ost = ostp.tile([C, CHUNK], f16, name=f"ost_{b}_{k}",
                                tag="ost")
                for j in range(CHUNK // OTILE):
                    op = ops.tile([C, OTILE], f32, name=f"op_{b}_{k}_{j}",
                                  tag="op")
                    nc.tensor.matmul(
                        op[:], attnT[:],
                        xb16[b][k][:, j * OTILE:(j + 1) * OTILE],
                        start=True, stop=True)
                    dst = ost[:, j * OTILE:(j + 1) * OTILE]
                    jj = k * (CHUNK // OTILE) + j
                    if jj % 2 == 0:
                        nc.vector.tensor_copy(dst, op[:])
                    else:
                        nc.scalar.copy(dst, op[:])
                nc.sync.dma_start(o_d[b, :, k * CHUNK:(k + 1) * CHUNK],
                                  ost[:])

            for b in range(B):
                emit_phase1_mms(b)   # ends with this batch's bounce + AR
            # phase-2 operand loads ride the HWDGE FIFO behind ALL phase-1
            # chunk loads: they use the otherwise-idle bandwidth while the
            # collectives run, and cannot delay the energy inputs.
            for b in range(B):
                for k in range(NCHUNK):
                    nc.sync.dma_start(xb16[b][k][:],
                                      xb_d[b, :, k * CHUNK:(k + 1) * CHUNK])
            for b in range(B):
                nc.gpsimd.dma_start(e_red[:, b * C:(b + 1) * C],
                                    ar_out[b][:])

            P0 = emit_softmax_vec(0)
            attnT0 = emit_softmax_fin(0, P0, nc.vector.tensor_copy)
            for k in range(5):
                emit_apply_chunk(0, attnT0, k)
            # batch 1 softmax rides the vector queue here so it executes
            # right as AR(b1) lands, between batch-0 output copies
            P1 = emit_softmax_vec(1)
            for k in range(5, NCHUNK):
                emit_apply_chunk(0, attnT0, k)
            attnT1 = emit_softmax_fin(1, P1, nc.scalar.copy)
            for k in range(NCHUNK):
                emit_apply_chunk(1, attnT1, k)

    _log("tile context done; bacc compile start")
    nc.compile()
    _log("bacc compile done")
    return nc


def _get_nc():
    if "nc" not in _compiled:
        _compiled["nc"] = _build()
    return _compiled["nc"]


def kernel(x, gamma, _trace=False, _tmpdir=None):
    import ml_dtypes
    from concourse import bass_utils

    bf16 = ml_dtypes.bfloat16
    x = np.ascontiguousarray(np.asarray(x), dtype=np.float32)
    gamma = np.asarray(gamma, dtype=np.float32)
    q = x.reshape(B, C, N)
    hi = q.astype(bf16)
    lo = (q - hi.astype(np.float32)).astype(bf16)
    # tile-major transposed layout: A[r][b, p, t, c] = qT[b, r*NLOC+t*128+p, c]
    Ahi = np.ascontiguousarray(
        hi.reshape(B, C, NCORES, T, C).transpose(2, 0, 4, 3, 1)
    ).reshape(NCORES, B, C, T * C)
    Alo = np.ascontiguousarray(
        lo.reshape(B, C, NCORES, T, C).transpose(2, 0, 4, 3, 1)
    ).reshape(NCORES, B, C, T * C)
    # normal-layout bf16 q for phase 2, sliced per core
    Axb = np.ascontiguousarray(
        hi.reshape(B, C, NCORES, NLOC).transpose(2, 0, 1, 3))
    gcol = np.full((C, 1), gamma[0], dtype=np.float32)
    ident = np.eye(C, dtype=np.float32)

    in_maps = []
    for r in range(NCORES):
        in_maps.append({
            "qhT": Ahi[r],
            "qlT": Alo[r],
            "xb": Axb[r],
            "gamma_col": gcol,
            "ident": ident,
        })

    nc = _get_nc()
    _log("launching run_bass_kernel_spmd")
    res = bass_utils.run_bass_kernel_spmd(
        nc, in_maps, core_ids=list(range(NCORES)), trace=_trace,
        tmpdir=_tmpdir)
    outs = [res.results[r]["out"] for r in range(NCORES)]
    full = np.concatenate(outs, axis=2).astype(np.float32)
    full = full.reshape(B, C, D, H, W)
    if _trace:
        return full.astype(np.float32, copy=False), res
    return full.astype(np.float32, copy=False)


# revision 23
# speedup vs baseline: 1.0663x; 1.0452x over previous
"""Channel-attention module kernel for 8 Trainium2 NeuronCores.

reference semantics (B=2, C=128, N=D*H*W=147456):
    q = x.reshape(B, C, N)
    energy = q @ q^T                  # [B, C, C]
    attn = softmax(rowmax(energy) - energy, axis=-1)
          = softmax(-energy, axis=-1)             (rowmax shift is a no-op)
    out = attn @ q
    return x + gamma * out

Sharding: sequence-parallel over N. Core r owns columns
[r*N/8, (r+1)*N/8) of q for both batches. Each core computes a partial
energy (contraction over its local n), per-batch AllReduces sum the tiny
[C, C] energies across the 8 cores, each core then computes the softmax
redundantly and applies the attention to its local columns.

Precision/layout scheme (v6):
  - The host splits q into q = hi + lo with hi = bf16(q) and
    lo = bf16(q - hi) (~16 mantissa bits combined), shipped in a
    pre-transposed tile-major layout A[b, p, t, c] = qT[b, t*128+p, c]
    so each [n=128, C] matmul operand tile is a plain column slice of a
    contiguous chunk DMA. energy = Qhi Qhi^T + (Qhi Qlo^T) +
    (Qhi Qlo^T)^T, dropping the O(2^-16) lo*lo term: two bf16 matmul
    chains per batch (1 cyc/row) instead of one fp32 chain (4 cyc/row).
    Measured pipeline rel err 1.7e-3 (phase-2 bf16 dominates).
  - phase 2 needs q in [C, n] layout: shipped as a THIRD host tensor
    (bf16, normal layout) rather than PE-transposing on device — the
    extra 9.4 MB of DMA streams through the otherwise-idle bandwidth
    window between the phase-1 loads and the output stores (HWDGE FIFO
    order guarantees it cannot delay the phase-1 loads), and it frees
    the PE of 288 transposes and DVE/ACT of 36 large PSUM copies.
  - phase 2 folds the residual into the attention matrix
    (attn_s = gamma/Z * P + I; P's diagonal is exactly 0), so
    out = attn_s @ q_hi in bf16, f16 out.

Collective path (hw-measured): the first collective pays a ~45-60us
ncfw cold-start from its dispatch trigger (trigger floor ~20us after
kernel start), so a dummy warmup AllReduce with NO input dependencies
is dispatched as the first gpsimd instruction (reading uninitialized
dram — its value is never used). The per-batch energy AllReduces then
run back-to-back on the warm CC engine; batch 0's softmax+apply hide
batch 1's AllReduce latency.
"""

import sys

sys.path.insert(0, "/opt/trn_rl_repo")

import numpy as np

B, C = 2, 128
D, H, W = 16, 96, 96
N = D * H * W  # 147456
NCORES = 8
NLOC = N // NCORES  # 18432
T = NLOC // C  # 144 n-tiles of 128 per batch
CHUNK = 2048
NCHUNK = NLOC // CHUNK  # 9
TPC = CHUNK // C  # 16 n-tiles per chunk
OTILE = 512

_compiled = {}


def _log(msg):
    import time as _t
    print(f"[kernel {_t.strftime('%H:%M:%S')}] {msg}", flush=True)


def _build():
    import concourse.bacc as bacc
    import concourse.tile as tile
    import concourse.mybir as mybir

    _log("build start")

    f32 = mybir.dt.float32
    f16 = mybir.dt.float16
    bf16 = mybir.dt.bfloat16
    nc = bacc.Bacc("TRN2", target_bir_lowering=False, debug=False,
                   num_devices=NCORES)

    hi_d = nc.dram_tensor("qhT", [B, C, T * C], bf16, kind="ExternalInput").ap()
    lo_d = nc.dram_tensor("qlT", [B, C, T * C], bf16, kind="ExternalInput").ap()
    g_d = nc.dram_tensor("gamma_col", [C, 1], f32, kind="ExternalInput").ap()
    id_d = nc.dram_tensor("ident", [C, C], f32, kind="ExternalInput").ap()
    idb_d = nc.dram_tensor("identb", [C, C], bf16, kind="ExternalInput").ap()
    o_d = nc.dram_tensor("out", [B, C, NLOC], f16, kind="ExternalOutput").ap()

    with tile.TileContext(nc) as tc:
        with (
            tc.tile_pool(name="hring", bufs=NCHUNK + 1) as hp,
            tc.tile_pool(name="lring", bufs=4) as lp,
            tc.tile_pool(name="xb16", bufs=B * NCHUNK) as xbp,
            tc.tile_pool(name="tps", bufs=2, space="PSUM") as tps,
            tc.tile_pool(name="eps", bufs=2, space="PSUM") as eps,
            tc.tile_pool(name="ops", bufs=4, space="PSUM") as ops,
            tc.tile_pool(name="misc", bufs=1) as mp,
            tc.tile_pool(name="ost", bufs=3) as ostp,
            tc.tile_pool(name="dram", bufs=1, space="DRAM") as dramp,
        ):
            # Warm-up collective FIRST, with no input dependency: the value
            # is garbage and never read; its only job is to absorb the ncfw
            # cold-start while input DMAs and phase 1 run. Disjoint PAIRS
            # instead of the 8-core ring: every core still inits ncfw, but
            # the mesh is a single hop, so the CC engine frees ~15us sooner
            # for the real AllReduces.
            w_in = dramp.tile([C, 1], f32, name="w_in")
            w_out = dramp.tile([C, 1], f32, name="w_out")
            nc.gpsimd.collective_compute(
                "AllReduce", mybir.AluOpType.add,
                replica_groups=[[2 * i, 2 * i + 1] for i in range(NCORES // 2)],
                ins=[w_in.opt()], outs=[w_out.opt()],
            )

            ident = mp.tile([C, C], f32, name="ident_sb")
            identb = mp.tile([C, C], bf16, name="identb_sb")
            nc.sync.dma_start(identb[:], idb_d[:])
            nc.sync.dma_start(ident[:], id_d[:])
            # first chunk split so the PE starts after a quarter-chunk
            ht0 = hp.tile([C, CHUNK], bf16, name="h_0_0", tag="h")
            nc.sync.dma_start(ht0[:, 0:512], hi_d[0, :, 0:512])
            lt0 = lp.tile([C, CHUNK], bf16, name="l_0_0", tag="l")
            nc.sync.dma_start(lt0[:, 0:512], lo_d[0, :, 0:512])
            nc.sync.dma_start(ht0[:, 512:CHUNK], hi_d[0, :, 512:CHUNK])
            nc.sync.dma_start(lt0[:, 512:CHUNK], lo_d[0, :, 512:CHUNK])
            gcol = mp.tile([C, 1], f32, name="gcol")
            nc.sync.dma_start(gcol[:], g_d[:])

            xb16 = [[xbp.tile([C, CHUNK], bf16, name=f"xb_{b}_{k}", tag="xb")
                     for k in range(NCHUNK)] for b in range(B)]

            # per-batch AllReduce buffers
            ar_in = [dramp.tile([C, C], f32, name=f"ar_in{b}")
                     for b in range(B)]
            ar_out = [dramp.tile([C, C], f32, name=f"ar_out{b}",
                                 addr_space="Shared") for b in range(B)]
            e_red = mp.tile([C, 2 * C], f32, name="e_red")

            hkeep = {}  # live hi chunks of the current batch

            def emit_phase1_mms(b):
                e_main = eps.tile([C, C], f32, name=f"em{b}", tag="e")
                e_cross = eps.tile([C, C], f32, name=f"ec{b}", tag="e")
                for k in range(NCHUNK):
                    if b == 0 and k == 0:
                        ht, lt = ht0, lt0
                    else:
                        ht = hp.tile([C, CHUNK], bf16, name=f"h_{b}_{k}",
                                     tag="h")
                        nc.sync.dma_start(
                            ht[:], hi_d[b, :, k * CHUNK:(k + 1) * CHUNK])
                        lt = lp.tile([C, CHUNK], bf16, name=f"l_{b}_{k}",
                                     tag="l")
                        nc.sync.dma_start(
                            lt[:], lo_d[b, :, k * CHUNK:(k + 1) * CHUNK])
                    hkeep[k] = ht
                    if b == 0 and k == 0:
                        # consume the quarter-chunk first so the PE starts
                        # as early as possible during the DMA ramp
                        order = [("hh", j) for j in range(4)] \
                            + [("hl", j) for j in range(4)] \
                            + [p for j in range(4, TPC)
                               for p in (("hh", j), ("hl", j))]
                    else:
                        order = [p for j in range(TPC)
                                 for p in (("hh", j), ("hl", j))]
                    for kind, j in order:
                        t = k * TPC + j
                        hs = ht[:, j * C:(j + 1) * C]
                        if kind == "hh":
                            nc.tensor.matmul(e_main[:], hs, hs,
                                             start=(t == 0), stop=(t == T - 1))
                        else:
                            nc.tensor.matmul(e_cross[:], hs,
                                             lt[:, j * C:(j + 1) * C],
                                             start=(t == 0), stop=(t == T - 1))
                # E_partial = e_main + e_cross + e_cross^T
                ecr = mp.tile([C, C], f32, name=f"ecr{b}")
                nc.vector.tensor_copy(ecr[:], e_cross[:])
                tpc_ps = tps.tile([C, C], f32, name=f"tpc{b}", tag="tp")
                nc.tensor.transpose(tpc_ps[:], ecr[:], ident[:])
                e_sum = mp.tile([C, C], f32, name=f"esum{b}")
                nc.vector.tensor_tensor(e_sum[:], e_main[:], ecr[:],
                                        op=mybir.AluOpType.add)
                e_cat = mp.tile([C, C], f32, name=f"e_cat{b}")
                nc.vector.tensor_tensor(e_cat[:], e_sum[:], tpc_ps[:],
                                        op=mybir.AluOpType.add)
                # bounce the AllReduce input; SWDGE (gpsimd) so the HWDGE
                # FIFO of chunk loads isn't blocked
                nc.gpsimd.dma_start(ar_in[b][:], e_cat[:])
                nc.gpsimd.collective_compute(
                    "AllReduce", mybir.AluOpType.add,
                    replica_groups=[list(range(NCORES))],
                    ins=[ar_in[b].opt()], outs=[ar_out[b].opt()],
                )

            def emit_transposes(b):
                # hi tiles -> [C, n] bf16 resident chunks for phase 2;
                # emitted after the AR dispatch so they fill the PE while
                # the collective path is busy.
                cp = 0
                for k in range(NCHUNK):
                    ht = hkeep[k]
                    for g in range(TPC // 8):
                        tp = tps.tile([C, 8 * C], bf16,
                                      name=f"tp_{b}_{k}_{g}", tag="tp")
                        for u in range(8):
                            j = g * 8 + u
                            nc.tensor.transpose(tp[:, u * C:(u + 1) * C],
                                                ht[:, j * C:(j + 1) * C],
                                                identb[:])
                        dst = xb16[b][k][:, g * 8 * C:(g + 1) * 8 * C]
                        if cp % 2 == 0:
                            nc.vector.tensor_copy(dst, tp[:])
                        else:
                            nc.scalar.copy(dst, tp[:])
                        cp += 1
                hkeep.clear()

            def emit_softmax_vec(b):
                # everything up to P_b = gamma/Z * exp(min-E) + I
                E_b = e_red[:, b * C:(b + 1) * C]
                mcol = mp.tile([C, 1], f32, name=f"mcol{b}")
                nc.vector.tensor_reduce(mcol[:], E_b, axis=mybir.AxisListType.X,
                                        op=mybir.AluOpType.min)
                P_b = mp.tile([C, C], f32, name=f"P{b}")
                zcol = mp.tile([C, 1], f32, name=f"zcol{b}")
                # P = exp(min_row - E), zcol = rowsum(P); exponents <= 0.
                # P's diagonal is exp(min - ~+147000) == 0 exactly.
                nc.scalar.activation(P_b[:], E_b,
                                     mybir.ActivationFunctionType.Exp,
                                     bias=mcol[:], scale=-1.0,
                                     accum_out=zcol[:])
                rz = mp.tile([C, 1], f32, name=f"rz{b}")
                nc.vector.reciprocal(rz[:], zcol[:])
                scol = mp.tile([C, 1], f32, name=f"scol{b}")
                nc.vector.tensor_tensor(scol[:], rz[:], gcol[:],
                                        op=mybir.AluOpType.mult)
                # attn_s = (gamma/Z) * P + I  -> matmul computes x + gamma*attn@q
                nc.vector.tensor_scalar_mul(P_b[:], P_b[:], scol[:])
                nc.vector.tensor_add(P_b[:], P_b[:], ident[:])
                return P_b

            def emit_softmax_fin(b, P_b, copy_eng):
                tp2 = tps.tile([C, C], f32, name=f"tpP{b}", tag="tp")
                nc.tensor.transpose(tp2[:], P_b[:], ident[:])
                attnT = mp.tile([C, C], bf16, name=f"attnT{b}")
                copy_eng(attnT[:], tp2[:])  # fp32 psum -> bf16
                return attnT

            def emit_apply_chunk(b, attnT, k):
                ost = ostp.tile([C, CHUNK], f16, name=f"ost_{b}_{k}",
                                tag="ost")
                for j in range(CHUNK // OTILE):
                    op = ops.tile([C, OTILE], f32, name=f"op_{b}_{k}_{j}",
                                  tag="op")
                    nc.tensor.matmul(
                        op[:], attnT[:],
                        xb16[b][k][:, j * OTILE:(j + 1) * OTILE],
                        start=True, stop=True)
                    dst = ost[:, j * OTILE:(j + 1) * OTILE]
                    jj = k * (CHUNK // OTILE) + j
                    if jj % 2 == 0:
                        nc.vector.tensor_copy(dst, op[:])
                    else:
                        nc.scalar.copy(dst, op[:])
                nc.sync.dma_start(o_d[b, :, k * CHUNK:(k + 1) * CHUNK],
                                  ost[:])

            for b in range(B):
                emit_phase1_mms(b)   # ends with this batch's bounce + AR
                emit_transposes(b)   # PE work that overlaps the collective
            for b in range(B):
                nc.gpsimd.dma_start(e_red[:, b * C:(b + 1) * C],
                                    ar_out[b][:])

            P0 = emit_softmax_vec(0)
            attnT0 = emit_softmax_fin(0, P0, nc.vector.tensor_copy)
            for k in range(5):
                emit_apply_chunk(0, attnT0, k)
            # batch 1 softmax rides the vector queue here so it executes
            # right as AR(b1) lands, between batch-0 output copies
            P1 = emit_softmax_vec(1)
            for k in range(5, NCHUNK):
                emit_apply_chunk(0, attnT0, k)
            attnT1 = emit_softmax_fin(1, P1, nc.scalar.copy)
            for k in range(NCHUNK):
                emit_apply_chunk(1, attnT1, k)

    _log("tile context done; bacc compile start")
    nc.compile()
    _log("bacc compile done")
    return nc


def _get_nc():
    if "nc" not in _compiled:
        _compiled["nc"] = _build()
    return _compiled["nc"]


def kernel(x, gamma, _trace=False, _tmpdir=None):
    import ml_dtypes
    from concourse import bass_utils

    bf16 = ml_dtypes.bfloat16
    x = np.ascontiguousarray(np.asarray(x), dtype=np.float32)
    gamma = np.asarray(gamma, dtype=np.float32)
    q = x.reshape(B, C, N)
    hi = q.astype(bf16)
    lo = (q - hi.astype(np.float32)).astype(bf16)
    # tile-major transposed layout: A[r][b, p, t, c] = qT[b, r*NLOC+t*128+p, c]
    Ahi = np.ascontiguousarray(
        hi.reshape(B, C, NCORES, T, C).transpose(2, 0, 4, 3, 1)
    ).reshape(NCORES, B, C, T * C)
    Alo = np.ascontiguousarray(
        lo.reshape(B, C, NCORES, T, C).transpose(2, 0, 4, 3, 1)
    ).reshape(NCORES, B, C, T * C)
    gcol = np.full((C, 1), gamma[0], dtype=np.float32)
    ident = np.eye(C, dtype=np.float32)
    identb = np.eye(C, dtype=bf16)

    in_maps = []
    for r in range(NCORES):
        in_maps.append({
            "qhT": Ahi[r],
            "qlT": Alo[r],
            "gamma_col": gcol,
            "ident": ident,
            "identb": identb,
        })

    nc = _get_nc()
    _log("launching run_bass_kernel_spmd")
    res = bass_utils.run_bass_kernel_spmd(
        nc, in_maps, core_ids=list(range(NCORES)), trace=_trace,
        tmpdir=_tmpdir)
    outs = [res.results[r]["out"] for r in range(NCORES)]
    full = np.concatenate(outs, axis=2).astype(np.float32)
    full = full.reshape(B, C, D, H, W)
    if _trace:
        return full.astype(np.float32, copy=False), res
    return full.astype(np.float32, copy=False)
